# revision 2
# baseline (speedup 1.0000x reference)
"""Trainium2 Bass kernel for LoRALinear: out = x @ W^T + bias + 2*(x@A^T)@B^T.

Data-parallel over token rows (1024/core). Differences vs v1:
  - W-stationary: psum tile = out^T[o_tile 128, 512 tok] (1 bank), accumulated
    over 32 k-tiles; per (o, k) the same stationary W tile feeds both token
    halves back-to-back. No bias matmuls (64 fewer PE instructions than v1).
  - bias is a per-partition scalar in the out^T layout -> fused into the
    PSUM->SBUF eviction as tensor_scalar_add on DVE (off the PE path).
  - Head interleave: the first HEAD_O o-tiles run k-chunk by k-chunk behind
    the streaming x DMA so the PE starts after ~1 chunk instead of waiting
    for most of x (x DMA 8.4MB ~24us vs one o-pass ~14us).
  - W packed host-side so each o-tile DMA is one contiguous 1MB block;
    output written as outT [4096, 1024] f32 per core; host transposes.
"""

import numpy as np

import concourse.mybir as mybir
import concourse.tile as tile
from concourse import bacc, bass_utils

N_CORES = 8
B, S, D_IN, D_OUT, R = 4, 2048, 4096, 4096, 16
SCALING = 2.0
M_TOTAL = B * S              # 8192
M_CORE = M_TOTAL // N_CORES  # 1024
P = 128
KO = D_IN // P               # 32 contraction tiles
O_TILES = D_OUT // P         # 32 output-column tiles
NH = 2                       # token halves (psum bank = 512 f32)
MH = M_CORE // NH            # 512
XCH = 16                     # x DMA chunks
CKO = KO // XCH              # 2 k-tiles per chunk
HEAD_O = 3                   # o-tiles interleaved with the x stream
F16 = mybir.dt.float16
F32 = mybir.dt.float32


def build_nc(reps: int = 1, dt16=None):
    if dt16 is None:
        dt16 = F16
    nc = bacc.Bacc("TRN2", target_bir_lowering=False, debug=False,
                   num_devices=N_CORES)

    xT_d = nc.dram_tensor("xT", [D_IN, M_CORE], dt16, kind="ExternalInput")
    # w packed host-side per o-tile: w_pk[o*P+p, ko*P+q] = W_eff[o*P+q, ko*P+p]
    w_pk_d = nc.dram_tensor("w_pk", [D_OUT, D_IN], dt16, kind="ExternalInput")
    # bias packed: bias_pk[p, o] = bias[o*P+p]
    bias_d = nc.dram_tensor("bias_pk", [P, O_TILES], F32, kind="ExternalInput")
    outT_d = nc.dram_tensor("outT", [D_OUT, M_CORE], F32, kind="ExternalOutput")

    xT_r = xT_d.ap().rearrange("(ko p) m -> p ko m", p=P)     # [128, 32, 1024]
    w_pk_r = w_pk_d.ap().rearrange("(o p) (ko q) -> o p ko q", p=P, q=P)
    outT_r = outT_d.ap().rearrange("(o p) m -> o p m", p=P)   # [32, 128, 1024]

    with tile.TileContext(nc) as tc:
        with (
            tc.tile_pool(name="xp", bufs=2) as x_pool,
            tc.tile_pool(name="wp", bufs=6) as w_pool,
            tc.tile_pool(name="cst", bufs=1) as c_pool,
            tc.tile_pool(name="op", bufs=4) as o_pool,
            tc.tile_pool(name="ps", bufs=8, space="PSUM") as ps_pool,
        ):
            def evict(o, h, ps, bias_sb):
                o_sb = o_pool.tile([P, MH], F32)
                nc.vector.tensor_scalar_add(o_sb[:], ps[:], bias_sb[:, o:o + 1])
                nc.sync.dma_start(outT_r[o, :, h * MH:(h + 1) * MH], o_sb[:])

            def body(_i=None):
                bias_sb = c_pool.tile([P, O_TILES], F32)
                x_sb = x_pool.tile([P, KO, M_CORE], dt16)
                w_tiles = []

                # Interleave w-tile and x-chunk DMAs on the queue so the
                # first matmul's deps (w0 + xc0) land ~2MB in, not after
                # the whole 8.4MB x transfer.
                def issue_x(i):
                    nc.sync.dma_start(
                        x_sb[:, i * CKO:(i + 1) * CKO, :],
                        xT_r[:, i * CKO:(i + 1) * CKO, :])

                # w0 in halves so the first matmul's deps are ~1MB in
                w0_sb = w_pool.tile([P, KO, P], dt16, name="w_sb", tag="w_sb")
                nc.sync.dma_start(w0_sb[:, :KO // 2, :], w_pk_r[0][:, :KO // 2, :])
                w_tiles.append(w0_sb)
                issue_x(0)
                nc.sync.dma_start(w0_sb[:, KO // 2:, :], w_pk_r[0][:, KO // 2:, :])
                for o in range(1, HEAD_O):
                    w_sb = w_pool.tile([P, KO, P], dt16, name="w_sb", tag="w_sb")
                    nc.sync.dma_start(w_sb[:], w_pk_r[o])
                    w_tiles.append(w_sb)
                    issue_x(o)
                for i in range(HEAD_O, XCH):
                    issue_x(i)
                nc.sync.dma_start(bias_sb[:], bias_d.ap())
                ps_tiles = [
                    [ps_pool.tile([P, MH], F32, name="ps", tag="ps")
                     for h in range(NH)]
                    for o in range(HEAD_O)]
                for kq in range(XCH):
                    for o in range(HEAD_O):
                        for h in range(NH):
                            for k in range(kq * CKO, (kq + 1) * CKO):
                                nc.tensor.matmul(
                                    ps_tiles[o][h][:],
                                    w_tiles[o][:, k, :],
                                    x_sb[:, k, h * MH:(h + 1) * MH],
                                    start=(k == 0), stop=(k == KO - 1))
                for o in range(HEAD_O):
                    for h in range(NH):
                        evict(o, h, ps_tiles[o][h], bias_sb)

                # steady state
                for o in range(HEAD_O, O_TILES):
                    w_sb = w_pool.tile([P, KO, P], dt16, name="w_sb", tag="w_sb")
                    nc.sync.dma_start(w_sb[:], w_pk_r[o])
                    for h in range(NH):
                        ps = ps_pool.tile([P, MH], F32, name="ps", tag="ps")
                        for k in range(KO):
                            nc.tensor.matmul(
                                ps[:], w_sb[:, k, :],
                                x_sb[:, k, h * MH:(h + 1) * MH],
                                start=(k == 0), stop=(k == KO - 1))
                        evict(o, h, ps, bias_sb)

            if reps == 1:
                body()
            else:
                with tc.For_i(0, reps, 1) as i:
                    body(i)

    nc.compile()
    return nc


_NC_CACHE = {}


def _get_nc(reps: int = 1, dt16=None):
    key = (reps, str(dt16))
    if key not in _NC_CACHE:
        _NC_CACHE[key] = build_nc(reps, dt16)
    return _NC_CACHE[key]


def prep_in_maps(x, weight, bias, lora_A, lora_B):
    xf = np.asarray(x, dtype=np.float32).reshape(M_TOTAL, D_IN)
    w_eff = np.asarray(weight, dtype=np.float32) + SCALING * (
        np.asarray(lora_B, dtype=np.float32) @ np.asarray(lora_A, dtype=np.float32))
    # w_pk[o*P+p, ko*P+q] = W_eff[o*P+q, ko*P+p]
    w_pk = np.ascontiguousarray(
        w_eff.reshape(O_TILES, P, KO, P).transpose(0, 3, 2, 1)
    ).astype(np.float16).reshape(D_OUT, D_IN)
    bias_pk = np.ascontiguousarray(
        np.asarray(bias, dtype=np.float32).reshape(O_TILES, P).T)
    in_maps = []
    for c in range(N_CORES):
        xT_c = np.ascontiguousarray(
            xf[c * M_CORE:(c + 1) * M_CORE].T).astype(np.float16)
        in_maps.append({"xT": xT_c, "w_pk": w_pk, "bias_pk": bias_pk})
    return in_maps


def kernel(x, weight, bias, lora_A, lora_B):
    nc = _get_nc(1)
    in_maps = prep_in_maps(x, weight, bias, lora_A, lora_B)
    res = bass_utils.run_bass_kernel_spmd(nc, in_maps, core_ids=list(range(N_CORES)))
    out = np.concatenate(
        [res.results[c]["outT"].T for c in range(N_CORES)], axis=0)
    return np.ascontiguousarray(out).reshape(B, S, D_OUT)
